# revision 17
# baseline (speedup 1.0000x reference)
"""Causal self-attention (RoPE) Trainium2 Bass kernel, tensor-parallel over heads.

Full problem: x[4,2048,2048] @ W_attn[2048,6144] -> qkv; RoPE(q,k); causal
softmax attention per head; y @ W_proj[2048,2048] -> out.
Returns (out, k, v) matching the reference:
  out [B,T,C], k [B,H,T,D] (post-RoPE), v [B,H,T,D].

Sharding: 16 heads / 8 cores = 2 heads per core.  Each core computes
qkv for its heads (768 cols of W_attn), attention, and a partial c_proj
(its 256 rows of W_proj); partial outputs are summed on the host.

Layout strategy: everything on-chip is kept feature-major ("transposed"):
  xT [C, B*T] (host pre-transposed), qkvT [768, B*T], yT [256, B*T],
  poutT [2048, B*T].
Scores are computed transposed (sT[k, q] = k @ q^T) so softmax reductions
over k become PE matmuls (ones-vector trick) and no attention transposes
are needed; v is PE-transposed once per (b, h).
"""

import numpy as np
import ml_dtypes
from contextlib import ExitStack

import concourse.bass as bass
import concourse.tile as tile
from concourse import bacc, mybir
from concourse.bass_utils import run_bass_kernel_spmd
F32 = mybir.dt.float32
F32R = mybir.dt.float32r
BF16 = mybir.dt.bfloat16
F16 = mybir.dt.float16
AF = mybir.ActivationFunctionType

N_HEAD = 16
C = 2048
D = 128
N_CORES = 8
HL = 2              # heads per core
FL = HL * D         # 256 local features
KC = C // 128       # 16 contraction tiles for qkv
NEG = -1e30
SCALE = 1.0 / np.sqrt(D)
LAST_RESULTS = None


def build_nc(B, T):
    """Build the per-core Bass program (SPMD: same program, different data)."""
    NT = B * T
    NB = NT // 512      # 512-token blocks over B*T
    QB = T // 512       # 512-token q blocks per batch
    KT = T // 128       # 128-token k tiles per batch
    TT = NT // 128      # 128-token tiles over B*T
    assert T % 512 == 0

    nc = bacc.Bacc(
        "TRN2", target_bir_lowering=False, debug=False, num_devices=N_CORES
    )

    # I/O
    xT = nc.dram_tensor("xT", [C, NT], BF16, kind="ExternalInput").ap()
    w_qkv = nc.dram_tensor("w_qkv", [C, 3 * FL], BF16, kind="ExternalInput").ap()
    b_qkv = nc.dram_tensor("b_qkv", [3 * FL, 1], F32, kind="ExternalInput").ap()
    w_proj = nc.dram_tensor("w_proj", [FL, C], BF16, kind="ExternalInput").ap()
    cosT = nc.dram_tensor("cosT", [D, T], F32, kind="ExternalInput").ap()
    sinT = nc.dram_tensor("sinT", [D, T], F32, kind="ExternalInput").ap()
    ident_in = nc.dram_tensor("ident", [128, 128], BF16, kind="ExternalInput").ap()
    ones_in = nc.dram_tensor("ones", [128, 1], BF16, kind="ExternalInput").ap()

    poutT = nc.dram_tensor("poutT", [C, NT], F16, kind="ExternalOutput").ap()
    kT_out = nc.dram_tensor("kT_out", [FL, NT], BF16, kind="ExternalOutput").ap()
    v_out = nc.dram_tensor("v_out", [B, HL, T, D], BF16, kind="ExternalOutput").ap()

    with tile.TileContext(nc) as tc, ExitStack() as ctx:
        dram = ctx.enter_context(tc.tile_pool(name="dram", bufs=1, space="DRAM"))
        qkT = dram.tile([2 * FL, NT], BF16)   # rows 0:256 q, 256:512 k
        yT = dram.tile([FL, NT], BF16)

        const = ctx.enter_context(tc.tile_pool(name="const", bufs=1))
        w_sb = const.tile([128, KC, 3 * FL], BF16)
        for k in range(KC):
            nc.sync.dma_start(w_sb[:, k, :], w_qkv[k * 128:(k + 1) * 128, :])
        wp_sb = const.tile([128, FL // 128, C], BF16)
        for k in range(FL // 128):
            nc.sync.dma_start(wp_sb[:, k, :], w_proj[k * 128:(k + 1) * 128, :])
        b_sb = const.tile([128, 3 * FL // 128], F32)
        for m in range(3 * FL // 128):
            nc.sync.dma_start(b_sb[:, m:m + 1], b_qkv[m * 128:(m + 1) * 128, :])
        cos_sb = const.tile([D, T], F32)
        nc.sync.dma_start(cos_sb, cosT)
        sin_sb = const.tile([D, T], F32)
        nc.sync.dma_start(sin_sb, sinT)
        ones_sb = const.tile([128, 1], BF16)
        nc.sync.dma_start(ones_sb, ones_in)
        ident = const.tile([128, 128], BF16)
        nc.sync.dma_start(ident, ident_in)

        # v kept SBUF-resident in natural [token, feature] layout all kernel
        v_all = const.tile([128, TT, FL], BF16)

        # ---- Phase 1: qkvT[m] = W_loc[:, m].T @ x.T (+bias; RoPE for q,k;
        #      PE-transpose for v straight into v_all)
        with tc.tile_pool(name="xt", bufs=2) as xt_pool, \
             tc.tile_pool(name="qkv_ps", bufs=4, space="PSUM") as qkv_ps, \
             tc.tile_pool(name="ps_t", bufs=2, space="PSUM") as ps_t, \
             tc.tile_pool(name="qkv_sb", bufs=6) as qkv_sb, \
             tc.tile_pool(name="rope", bufs=4) as rope_pool:
            for nb in range(NB):
                xt = xt_pool.tile([128, KC, 512], BF16, tag="xt")
                for k in range(KC):
                    nc.sync.dma_start(
                        xt[:, k, :], xT[k * 128:(k + 1) * 128, nb * 512:(nb + 1) * 512]
                    )
                tp = (nb % QB) * 512  # position offset within batch
                for m in range(6):
                    ps = qkv_ps.tile([128, 512], F32, tag="ps")
                    for k in range(KC):
                        nc.tensor.matmul(
                            ps,
                            w_sb[:, k, m * 128:(m + 1) * 128],
                            xt[:, k, :],
                            start=(k == 0), stop=(k == KC - 1),
                        )
                    if m < 4:
                        xb = qkv_sb.tile([128, 512], F32, tag="xb")
                        nc.vector.tensor_scalar_add(xb, ps, b_sb[:, m:m + 1])
                        # RoPE: out = xb * cos + rotate_half(xb) * sin
                        rh = rope_pool.tile([128, 512], F32, tag="rh")
                        nc.gpsimd.tensor_scalar_mul(rh[0:64, :], xb[64:128, :], -1.0)
                        nc.gpsimd.tensor_copy(rh[64:128, :], xb[0:64, :])
                        t1 = rope_pool.tile([128, 512], F32, tag="t1")
                        nc.vector.tensor_mul(t1, xb, cos_sb[:, tp:tp + 512])
                        nc.vector.tensor_mul(rh, rh, sin_sb[:, tp:tp + 512])
                        xr = qkv_sb.tile([128, 512], BF16, tag="xr")
                        nc.vector.tensor_add(xr, t1, rh)
                        nc.sync.dma_start(
                            qkT[m * 128:(m + 1) * 128, nb * 512:(nb + 1) * 512], xr
                        )
                        if m in (2, 3):
                            nc.sync.dma_start(
                                kT_out[(m - 2) * 128:(m - 1) * 128,
                                       nb * 512:(nb + 1) * 512],
                                xr,
                            )
                    else:
                        xb = qkv_sb.tile([128, 512], BF16, tag="xbv")
                        nc.vector.tensor_scalar_add(xb, ps, b_sb[:, m:m + 1])
                        hl = m - 4
                        for tj in range(4):
                            pst = ps_t.tile([128, 128], BF16, tag="pst")
                            nc.tensor.transpose(
                                pst, xb[:, tj * 128:(tj + 1) * 128], ident
                            )
                            nc.vector.tensor_copy(
                                v_all[:, nb * 4 + tj, hl * 128:(hl + 1) * 128], pst
                            )
            # v output: one batched DMA per (b, hl)
            for b in range(B):
                for hl in range(HL):
                    nc.sync.dma_start(
                        v_out[b, hl].rearrange("(k p) d -> p k d", p=128),
                        v_all[:, b * KT:(b + 1) * KT, hl * 128:(hl + 1) * 128],
                    )

        # ---- Phase 2: attention per (b, local head) ----
        with tc.tile_pool(name="att_in", bufs=2) as att_in, \
             tc.tile_pool(name="attn_sb", bufs=3) as attn_sb, \
             tc.tile_pool(name="y_sb", bufs=3) as y_sb, \
             tc.tile_pool(name="rs", bufs=2) as rs_pool, \
             tc.tile_pool(name="ps_s", bufs=3, space="PSUM") as ps_s, \
             tc.tile_pool(name="ps_o", bufs=2, space="PSUM") as ps_o, \
             tc.tile_pool(name="ps_sum", bufs=2, space="PSUM") as ps_sum:
            for b in range(B):
                for hl in range(HL):
                    col0 = b * T
                    qTt = att_in.tile([128, T], BF16, tag="qT")
                    nc.sync.dma_start(
                        qTt, qkT[hl * 128:(hl + 1) * 128, col0:col0 + T]
                    )
                    kTt = att_in.tile([128, T], BF16, tag="kT")
                    nc.sync.dma_start(
                        kTt, qkT[(2 + hl) * 128:(3 + hl) * 128, col0:col0 + T]
                    )
                    for qb in range(QB):
                        pso = ps_o.tile([128, 512], F32, tag="o")
                        pss = ps_sum.tile([1, 512], F32, tag="s")
                        kmax = 4 * qb + 3
                        for ki in range(kmax + 1):
                            # Diagonal blocks: columns left of the diagonal
                            # stripe are fully masked -- skip computing them.
                            jj = ki - 4 * qb
                            qoff = max(jj, 0) * 128
                            NP = 512 - qoff
                            ps = ps_s.tile([128, 512], F32, tag="sc")
                            nc.tensor.matmul(
                                ps[:, :NP],
                                kTt[:, ki * 128:(ki + 1) * 128],
                                qTt[:, qb * 512 + qoff:(qb + 1) * 512],
                                start=True, stop=True,
                            )
                            at = attn_sb.tile([128, 512], BF16, tag="at")
                            nc.scalar.activation(
                                at[:, :NP], ps[:, :NP], AF.Exp, scale=SCALE
                            )
                            if jj >= 0:
                                # zero the strictly-upper triangle of the
                                # diagonal 128x128 block (k > q -> 0)
                                nc.gpsimd.affine_select(
                                    out=at[:, 0:128], in_=at[:, 0:128],
                                    compare_op=mybir.AluOpType.is_ge, fill=0.0,
                                    base=0, pattern=[[1, 128]],
                                    channel_multiplier=-1,
                                )
                            nc.tensor.matmul(
                                pso[:, qoff:512],
                                v_all[:, b * KT + ki, hl * 128:(hl + 1) * 128],
                                at[:, :NP],
                                start=(ki == 0), stop=(ki == kmax),
                            )
                            nc.tensor.matmul(
                                pss[:, qoff:512], ones_sb, at[:, :NP],
                                start=(ki == 0), stop=(ki == kmax),
                            )
                        rs = rs_pool.tile([1, 512], F32, tag="rs")
                        nc.vector.reciprocal(rs, pss)
                        rbc = rs_pool.tile([128, 512], F32, tag="rbc")
                        nc.gpsimd.partition_broadcast(rbc, rs)
                        y = y_sb.tile([128, 512], BF16, tag="y")
                        nc.vector.tensor_mul(y, pso, rbc)
                        nc.sync.dma_start(
                            yT[hl * 128:(hl + 1) * 128,
                               col0 + qb * 512:col0 + (qb + 1) * 512],
                            y,
                        )

        # ---- Phase 3: poutT[m, nb] = W_proj_loc.T @ yT (partial; host sums) ----
        with tc.tile_pool(name="yin", bufs=2) as yin_pool, \
             tc.tile_pool(name="po_sb", bufs=4) as po_sb, \
             tc.tile_pool(name="proj_ps", bufs=4, space="PSUM") as proj_ps:
            for nb in range(NB):
                yt = yin_pool.tile([128, FL // 128, 512], BF16, tag="yt")
                for k in range(FL // 128):
                    nc.sync.dma_start(
                        yt[:, k, :], yT[k * 128:(k + 1) * 128, nb * 512:(nb + 1) * 512]
                    )
                for mp in range(C // 256):
                    po = po_sb.tile([128, 2, 512], F16, tag="po")
                    for mh in range(2):
                        m = mp * 2 + mh
                        ps = proj_ps.tile([128, 512], F32, tag="pp")
                        for k in range(FL // 128):
                            nc.tensor.matmul(
                                ps,
                                wp_sb[:, k, m * 128:(m + 1) * 128],
                                yt[:, k, :],
                                start=(k == 0), stop=(k == FL // 128 - 1),
                            )
                        nc.vector.tensor_copy(po[:, mh, :], ps)
                    dst = bass.AP(
                        tensor=poutT.tensor,
                        offset=mp * 256 * NT + nb * 512,
                        ap=[[NT, 128], [128 * NT, 2], [1, 512]],
                    )
                    nc.sync.dma_start(dst, po)

    nc.compile()
    return nc


def host_rope_tables(T):
    inv_freq = 1.0 / (10000.0 ** (np.arange(0, D, 2, dtype=np.float32) / D))
    pos = np.arange(T, dtype=np.float32)
    freqs = np.outer(pos, inv_freq)                       # [T, D/2]
    emb = np.concatenate([freqs, freqs], axis=-1)         # [T, D]
    return (
        np.ascontiguousarray(np.cos(emb).astype(np.float32).T),  # [D, T]
        np.ascontiguousarray(np.sin(emb).astype(np.float32).T),
    )


def _ensure_ntff_hook():
    """Install the axon NTFF profile hook if the image's antenv lacks it.

    Only needed for BASS_TRACE=1 timing runs; guarded so grading runs are
    unaffected if anything here is missing."""
    import sys, types
    try:
        from antenv.axon_hooks import get_axon_ntff_profile_hook  # noqa: F401
        return
    except ImportError:
        pass
    try:
        import antenv
        from trn_agent_boot.trn_boot import _ntff_profile_via_ctypes
        mod = types.ModuleType("antenv.axon_hooks")
        state = {"hook": _ntff_profile_via_ctypes("/opt/axon/libaxon_pjrt.so")}
        mod.get_axon_ntff_profile_hook = lambda: state["hook"]
        mod.set_axon_ntff_profile_hook = lambda h: state.__setitem__("hook", h)
        sys.modules["antenv.axon_hooks"] = mod
        antenv.axon_hooks = mod
    except Exception:
        pass
    try:
        from concourse import bass_utils as _bu
        _orig = _bu.upload_artifacts
        def _safe_upload(tmpdir):
            try:
                return _orig(tmpdir)
            except Exception:
                return tmpdir
        _bu.upload_artifacts = _safe_upload
    except Exception:
        pass


def kernel(x, W_attn, b_attn, W_proj, b_proj):
    x = np.asarray(x, dtype=np.float32)
    W_attn = np.asarray(W_attn, dtype=np.float32)
    b_attn = np.asarray(b_attn, dtype=np.float32)
    W_proj = np.asarray(W_proj, dtype=np.float32)
    b_proj = np.asarray(b_proj, dtype=np.float32)

    B, T, _ = x.shape
    NT = B * T
    nc = build_nc(B, T)

    xT = np.ascontiguousarray(x.reshape(NT, C).T).astype(ml_dtypes.bfloat16)
    cosT, sinT = host_rope_tables(T)

    in_maps = []
    for c in range(N_CORES):
        f0 = c * FL
        cols = np.r_[f0:f0 + FL, C + f0:C + f0 + FL, 2 * C + f0:2 * C + f0 + FL]
        in_maps.append({
            "xT": xT,
            "w_qkv": np.ascontiguousarray(W_attn[:, cols]).astype(ml_dtypes.bfloat16),
            "b_qkv": np.ascontiguousarray(b_attn[cols]).reshape(3 * FL, 1),
            "w_proj": np.ascontiguousarray(W_proj[f0:f0 + FL, :]).astype(ml_dtypes.bfloat16),
            "cosT": cosT,
            "sinT": sinT,
            "ident": np.eye(128).astype(ml_dtypes.bfloat16),
            "ones": np.ones((128, 1)).astype(ml_dtypes.bfloat16),
        })

    import os as _os
    if _os.environ.get("BASS_TRACE"):
        _ensure_ntff_hook()
    res = run_bass_kernel_spmd(nc, in_maps, core_ids=list(range(N_CORES)))
    global LAST_RESULTS
    LAST_RESULTS = res

    out_T = np.zeros((C, NT), dtype=np.float32)
    ks, vs = [], []
    for c in range(N_CORES):
        r = res.results[c]
        out_T += r["poutT"].astype(np.float32)
        kc = (r["kT_out"].astype(np.float32)
              .reshape(HL, D, B, T).transpose(2, 0, 3, 1))  # [B,HL,T,D]
        ks.append(kc)
        vs.append(r["v_out"].astype(np.float32))
    out = out_T.T + b_proj[None, :]
    out = np.ascontiguousarray(out.reshape(B, T, C), dtype=np.float32)
    k = np.ascontiguousarray(np.concatenate(ks, axis=1), dtype=np.float32)
    v = np.ascontiguousarray(np.concatenate(vs, axis=1), dtype=np.float32)
    return out, k, v


# revision 21
# speedup vs baseline: 1.6477x; 1.6477x over previous
"""Causal self-attention (RoPE) Trainium2 Bass kernel, tensor-parallel over heads.

Full problem: x[4,2048,2048] @ W_attn[2048,6144] -> qkv; RoPE(q,k); causal
softmax attention per head; y @ W_proj[2048,2048] -> out.
Returns (out, k, v) matching the reference:
  out [B,T,C], k [B,H,T,D] (post-RoPE), v [B,H,T,D].

Sharding: 16 heads / 8 cores = 2 heads per core.  Each core computes
qkv for its heads (768 cols of W_attn), attention, and a partial c_proj
(its 256 rows of W_proj); partial outputs are summed on the host.

Layout strategy: everything on-chip is kept feature-major ("transposed"):
  xT [C, B*T] (host pre-transposed), qkvT [768, B*T], yT [256, B*T],
  poutT [2048, B*T].
Scores are computed transposed (sT[k, q] = k @ q^T) so softmax reductions
over k become PE matmuls (ones-vector trick) and no attention transposes
are needed; v is PE-transposed once per (b, h).
"""

import numpy as np
import ml_dtypes
from contextlib import ExitStack

import concourse.bass as bass
import concourse.tile as tile
from concourse import bacc, mybir
from concourse.bass_utils import run_bass_kernel_spmd
F32 = mybir.dt.float32
F32R = mybir.dt.float32r
BF16 = mybir.dt.bfloat16
F16 = mybir.dt.float16
AF = mybir.ActivationFunctionType

N_HEAD = 16
C = 2048
D = 128
N_CORES = 8
HL = 2              # heads per core
FL = HL * D         # 256 local features
KC = C // 128       # 16 contraction tiles for qkv
NEG = -1e30
SCALE = 1.0 / np.sqrt(D)
LAST_RESULTS = None


def build_nc(B, T):
    """Build the per-core Bass program (SPMD: same program, different data)."""
    NT = B * T
    NB = NT // 512      # 512-token blocks over B*T
    QB = T // 512       # 512-token q blocks per batch
    KT = T // 128       # 128-token k tiles per batch
    TT = NT // 128      # 128-token tiles over B*T
    assert T % 512 == 0

    nc = bacc.Bacc(
        "TRN2", target_bir_lowering=False, debug=False, num_devices=N_CORES
    )

    # I/O
    xT = nc.dram_tensor("xT", [C, NT], BF16, kind="ExternalInput").ap()
    w_qkv = nc.dram_tensor("w_qkv", [C, 3 * FL], BF16, kind="ExternalInput").ap()
    b_qkv = nc.dram_tensor("b_qkv", [3 * FL, 1], F32, kind="ExternalInput").ap()
    w_proj = nc.dram_tensor("w_proj", [FL, C], BF16, kind="ExternalInput").ap()
    cosT = nc.dram_tensor("cosT", [D, T], F32, kind="ExternalInput").ap()
    sinT = nc.dram_tensor("sinT", [D, T], F32, kind="ExternalInput").ap()
    ident_in = nc.dram_tensor("ident", [128, 128], BF16, kind="ExternalInput").ap()
    ones_in = nc.dram_tensor("ones", [128, 1], BF16, kind="ExternalInput").ap()

    poutT = nc.dram_tensor("poutT", [C, NT], F16, kind="ExternalOutput").ap()
    kT_out = nc.dram_tensor("kT_out", [FL, NT], BF16, kind="ExternalOutput").ap()
    v_out = nc.dram_tensor("v_out", [B, HL, T, D], BF16, kind="ExternalOutput").ap()

    with tile.TileContext(nc) as tc, ExitStack() as ctx:
        dram = ctx.enter_context(tc.tile_pool(name="dram", bufs=1, space="DRAM"))
        qkT = dram.tile([2 * FL, NT], BF16)   # rows 0:256 q, 256:512 k
        yT = dram.tile([FL, NT], BF16)

        const = ctx.enter_context(tc.tile_pool(name="const", bufs=1))
        w_sb = const.tile([128, KC, 3 * FL], BF16)
        for k in range(KC):
            nc.sync.dma_start(w_sb[:, k, :], w_qkv[k * 128:(k + 1) * 128, :])
        wp_sb = const.tile([128, FL // 128, C], BF16)
        for k in range(FL // 128):
            nc.sync.dma_start(wp_sb[:, k, :], w_proj[k * 128:(k + 1) * 128, :])
        b_sb = const.tile([128, 3 * FL // 128], F32)
        for m in range(3 * FL // 128):
            nc.sync.dma_start(b_sb[:, m:m + 1], b_qkv[m * 128:(m + 1) * 128, :])
        cos_sb = const.tile([D, T], F32)
        nc.sync.dma_start(cos_sb, cosT)
        sin_sb = const.tile([D, T], F32)
        nc.sync.dma_start(sin_sb, sinT)
        ones_sb = const.tile([128, 1], BF16)
        nc.sync.dma_start(ones_sb, ones_in)
        ident = const.tile([128, 128], BF16)
        nc.sync.dma_start(ident, ident_in)

        # v kept SBUF-resident in natural [token, feature] layout all kernel
        v_all = const.tile([128, TT, FL], BF16)

        # ---- Phase 1: qkvT[m] = W_loc[:, m].T @ x.T (+bias; RoPE for q,k;
        #      PE-transpose for v straight into v_all)
        with tc.tile_pool(name="xt", bufs=2) as xt_pool, \
             tc.tile_pool(name="qkv_ps", bufs=4, space="PSUM") as qkv_ps, \
             tc.tile_pool(name="ps_t", bufs=2, space="PSUM") as ps_t, \
             tc.tile_pool(name="qkv_sb", bufs=6) as qkv_sb, \
             tc.tile_pool(name="rope", bufs=4) as rope_pool:
            for nb in range(NB):
                xt = xt_pool.tile([128, KC, 512], BF16, tag="xt")
                for k in range(KC):
                    nc.sync.dma_start(
                        xt[:, k, :], xT[k * 128:(k + 1) * 128, nb * 512:(nb + 1) * 512]
                    )
                tp = (nb % QB) * 512  # position offset within batch
                for m in range(6):
                    ps = qkv_ps.tile([128, 512], F32, tag="ps")
                    for k in range(KC):
                        nc.tensor.matmul(
                            ps,
                            w_sb[:, k, m * 128:(m + 1) * 128],
                            xt[:, k, :],
                            start=(k == 0), stop=(k == KC - 1),
                        )
                    if m < 4:
                        xb = qkv_sb.tile([128, 512], F32, tag="xb")
                        nc.vector.tensor_scalar_add(xb, ps, b_sb[:, m:m + 1])
                        # RoPE: out = xb * cos + rotate_half(xb) * sin
                        rh = rope_pool.tile([128, 512], F32, tag="rh")
                        nc.vector.tensor_scalar_mul(rh[0:64, :], xb[64:128, :], -1.0)
                        nc.vector.tensor_copy(rh[64:128, :], xb[0:64, :])
                        t1 = rope_pool.tile([128, 512], F32, tag="t1")
                        nc.vector.tensor_mul(t1, xb, cos_sb[:, tp:tp + 512])
                        nc.vector.tensor_mul(rh, rh, sin_sb[:, tp:tp + 512])
                        xr = qkv_sb.tile([128, 512], BF16, tag="xr")
                        nc.vector.tensor_add(xr, t1, rh)
                        nc.sync.dma_start(
                            qkT[m * 128:(m + 1) * 128, nb * 512:(nb + 1) * 512], xr
                        )
                        if m in (2, 3):
                            nc.sync.dma_start(
                                kT_out[(m - 2) * 128:(m - 1) * 128,
                                       nb * 512:(nb + 1) * 512],
                                xr,
                            )
                    else:
                        xb = qkv_sb.tile([128, 512], BF16, tag="xbv")
                        nc.vector.tensor_scalar_add(xb, ps, b_sb[:, m:m + 1])
                        hl = m - 4
                        for tj in range(4):
                            pst = ps_t.tile([128, 128], BF16, tag="pst")
                            nc.tensor.transpose(
                                pst, xb[:, tj * 128:(tj + 1) * 128], ident
                            )
                            nc.vector.tensor_copy(
                                v_all[:, nb * 4 + tj, hl * 128:(hl + 1) * 128], pst
                            )
            # v output: one batched DMA per (b, hl)
            for b in range(B):
                for hl in range(HL):
                    nc.sync.dma_start(
                        v_out[b, hl].rearrange("(k p) d -> p k d", p=128),
                        v_all[:, b * KT:(b + 1) * KT, hl * 128:(hl + 1) * 128],
                    )

        # ---- Phase 2: attention per (b, local head) ----
        with tc.tile_pool(name="att_in", bufs=2) as att_in, \
             tc.tile_pool(name="attn_sb", bufs=3) as attn_sb, \
             tc.tile_pool(name="y_sb", bufs=3) as y_sb, \
             tc.tile_pool(name="rs", bufs=2) as rs_pool, \
             tc.tile_pool(name="ps_s", bufs=3, space="PSUM") as ps_s, \
             tc.tile_pool(name="ps_o", bufs=2, space="PSUM") as ps_o, \
             tc.tile_pool(name="ps_sum", bufs=2, space="PSUM") as ps_sum:
            for b in range(B):
                for hl in range(HL):
                    col0 = b * T
                    qTt = att_in.tile([128, T], BF16, tag="qT")
                    nc.sync.dma_start(
                        qTt, qkT[hl * 128:(hl + 1) * 128, col0:col0 + T]
                    )
                    kTt = att_in.tile([128, T], BF16, tag="kT")
                    nc.sync.dma_start(
                        kTt, qkT[(2 + hl) * 128:(3 + hl) * 128, col0:col0 + T]
                    )
                    for qb in range(QB):
                        pso = ps_o.tile([128, 512], F32, tag="o")
                        pss = ps_sum.tile([1, 512], F32, tag="s")
                        kmax = 4 * qb + 3
                        for ki in range(kmax + 1):
                            # Diagonal blocks: columns left of the diagonal
                            # stripe are fully masked -- skip computing them.
                            jj = ki - 4 * qb
                            qoff = max(jj, 0) * 128
                            NP = 512 - qoff
                            ps = ps_s.tile([128, 512], F32, tag="sc")
                            nc.tensor.matmul(
                                ps[:, :NP],
                                kTt[:, ki * 128:(ki + 1) * 128],
                                qTt[:, qb * 512 + qoff:(qb + 1) * 512],
                                start=True, stop=True,
                            )
                            at = attn_sb.tile([128, 512], BF16, tag="at")
                            nc.scalar.activation(
                                at[:, :NP], ps[:, :NP], AF.Exp, scale=SCALE
                            )
                            if jj >= 0:
                                # zero the strictly-upper triangle of the
                                # diagonal 128x128 block (k > q -> 0)
                                nc.gpsimd.affine_select(
                                    out=at[:, 0:128], in_=at[:, 0:128],
                                    compare_op=mybir.AluOpType.is_ge, fill=0.0,
                                    base=0, pattern=[[1, 128]],
                                    channel_multiplier=-1,
                                )
                            nc.tensor.matmul(
                                pso[:, qoff:512],
                                v_all[:, b * KT + ki, hl * 128:(hl + 1) * 128],
                                at[:, :NP],
                                start=(ki == 0), stop=(ki == kmax),
                            )
                            nc.tensor.matmul(
                                pss[:, qoff:512], ones_sb, at[:, :NP],
                                start=(ki == 0), stop=(ki == kmax),
                            )
                        rs = rs_pool.tile([1, 512], F32, tag="rs")
                        nc.vector.reciprocal(rs, pss)
                        rbc = rs_pool.tile([128, 512], F32, tag="rbc")
                        nc.gpsimd.partition_broadcast(rbc, rs)
                        y = y_sb.tile([128, 512], BF16, tag="y")
                        nc.vector.tensor_mul(y, pso, rbc)
                        nc.sync.dma_start(
                            yT[hl * 128:(hl + 1) * 128,
                               col0 + qb * 512:col0 + (qb + 1) * 512],
                            y,
                        )

        # ---- Phase 3: poutT[m, nb] = W_proj_loc.T @ yT (partial; host sums) ----
        with tc.tile_pool(name="yin", bufs=2) as yin_pool, \
             tc.tile_pool(name="po_sb", bufs=4) as po_sb, \
             tc.tile_pool(name="proj_ps", bufs=4, space="PSUM") as proj_ps:
            for nb in range(NB):
                yt = yin_pool.tile([128, FL // 128, 512], BF16, tag="yt")
                for k in range(FL // 128):
                    nc.sync.dma_start(
                        yt[:, k, :], yT[k * 128:(k + 1) * 128, nb * 512:(nb + 1) * 512]
                    )
                for mp in range(C // 256):
                    po = po_sb.tile([128, 2, 512], F16, tag="po")
                    for mh in range(2):
                        m = mp * 2 + mh
                        ps = proj_ps.tile([128, 512], F32, tag="pp")
                        for k in range(FL // 128):
                            nc.tensor.matmul(
                                ps,
                                wp_sb[:, k, m * 128:(m + 1) * 128],
                                yt[:, k, :],
                                start=(k == 0), stop=(k == FL // 128 - 1),
                            )
                        nc.vector.tensor_copy(po[:, mh, :], ps)
                    dst = poutT[mp * 256:(mp + 1) * 256,
                                nb * 512:(nb + 1) * 512].rearrange(
                        "(a p) n -> p a n", p=128
                    )
                    nc.sync.dma_start(dst, po)

    nc.compile()
    return nc


def host_rope_tables(T):
    inv_freq = 1.0 / (10000.0 ** (np.arange(0, D, 2, dtype=np.float32) / D))
    pos = np.arange(T, dtype=np.float32)
    freqs = np.outer(pos, inv_freq)                       # [T, D/2]
    emb = np.concatenate([freqs, freqs], axis=-1)         # [T, D]
    return (
        np.ascontiguousarray(np.cos(emb).astype(np.float32).T),  # [D, T]
        np.ascontiguousarray(np.sin(emb).astype(np.float32).T),
    )


def _ensure_ntff_hook():
    """Install the axon NTFF profile hook if the image's antenv lacks it.

    Only needed for BASS_TRACE=1 timing runs; guarded so grading runs are
    unaffected if anything here is missing."""
    import sys, types
    try:
        from antenv.axon_hooks import get_axon_ntff_profile_hook  # noqa: F401
        return
    except ImportError:
        pass
    try:
        import antenv
        from trn_agent_boot.trn_boot import _ntff_profile_via_ctypes
        mod = types.ModuleType("antenv.axon_hooks")
        state = {"hook": _ntff_profile_via_ctypes("/opt/axon/libaxon_pjrt.so")}
        mod.get_axon_ntff_profile_hook = lambda: state["hook"]
        mod.set_axon_ntff_profile_hook = lambda h: state.__setitem__("hook", h)
        sys.modules["antenv.axon_hooks"] = mod
        antenv.axon_hooks = mod
    except Exception:
        pass
    try:
        from concourse import bass_utils as _bu
        _orig = _bu.upload_artifacts
        def _safe_upload(tmpdir):
            try:
                return _orig(tmpdir)
            except Exception:
                return tmpdir
        _bu.upload_artifacts = _safe_upload
    except Exception:
        pass


def kernel(x, W_attn, b_attn, W_proj, b_proj):
    x = np.asarray(x, dtype=np.float32)
    W_attn = np.asarray(W_attn, dtype=np.float32)
    b_attn = np.asarray(b_attn, dtype=np.float32)
    W_proj = np.asarray(W_proj, dtype=np.float32)
    b_proj = np.asarray(b_proj, dtype=np.float32)

    B, T, _ = x.shape
    NT = B * T
    nc = build_nc(B, T)

    xT = np.ascontiguousarray(x.reshape(NT, C).T).astype(ml_dtypes.bfloat16)
    cosT, sinT = host_rope_tables(T)

    in_maps = []
    for c in range(N_CORES):
        f0 = c * FL
        cols = np.r_[f0:f0 + FL, C + f0:C + f0 + FL, 2 * C + f0:2 * C + f0 + FL]
        in_maps.append({
            "xT": xT,
            "w_qkv": np.ascontiguousarray(W_attn[:, cols]).astype(ml_dtypes.bfloat16),
            "b_qkv": np.ascontiguousarray(b_attn[cols]).reshape(3 * FL, 1),
            "w_proj": np.ascontiguousarray(W_proj[f0:f0 + FL, :]).astype(ml_dtypes.bfloat16),
            "cosT": cosT,
            "sinT": sinT,
            "ident": np.eye(128).astype(ml_dtypes.bfloat16),
            "ones": np.ones((128, 1)).astype(ml_dtypes.bfloat16),
        })

    import os as _os
    if _os.environ.get("BASS_TRACE"):
        _ensure_ntff_hook()
    res = run_bass_kernel_spmd(nc, in_maps, core_ids=list(range(N_CORES)))
    global LAST_RESULTS
    LAST_RESULTS = res

    out_T = np.zeros((C, NT), dtype=np.float32)
    ks, vs = [], []
    for c in range(N_CORES):
        r = res.results[c]
        out_T += r["poutT"].astype(np.float32)
        kc = (r["kT_out"].astype(np.float32)
              .reshape(HL, D, B, T).transpose(2, 0, 3, 1))  # [B,HL,T,D]
        ks.append(kc)
        vs.append(r["v_out"].astype(np.float32))
    out = out_T.T + b_proj[None, :]
    out = np.ascontiguousarray(out.reshape(B, T, C), dtype=np.float32)
    k = np.ascontiguousarray(np.concatenate(ks, axis=1), dtype=np.float32)
    v = np.ascontiguousarray(np.concatenate(vs, axis=1), dtype=np.float32)
    return out, k, v
